# revision 20
# baseline (speedup 1.0000x reference)
"""Trainium2 Bass kernel for nn_DescriptorMatch (retrieval_knn).

Pipeline: bilinear-sample src keypoints -> ZNCC normalize -> [B,N,HW]
correlation vs the (ZNCC-normalized) full target feature map -> temp-0.01
softmax over HW -> soft-argmax to pseudo keypoints -> re-sample -> weights.

Sharding: 8 cores = (batch b in {0,1}) x (4-way split of the HW axis).
Each core handles all N=256 src keypoints against a contiguous slice of
49152 target positions (96 image rows), writing its slice of match_vals
and returning softmax partial sums (fixed-bound exp trick, so partials
from different cores combine by plain addition on the host).

Device work per 512-column chunk (one target image row):
  1. DMA F tile [c=128, 512] fp32
  2. sigma^-1 broadcast: one K=8 fp16 matmul recombines host-prepared
     (hi, lo*2^10) fp16 rows through a one-hot selector whose lo-rows are
     2^-10 -> exact fp32 sigma to ~1e-7, materialized across partitions
  3. F_norm = F * sigma_bcast (DVE; PSUM operand)
  4. main fp32 matmuls vs desc_src (2 row-groups of 128 keypoints)
     -> PSUM holds final match_vals = desc_src_norm^T @ F_trg_norm / C
  5. evict PSUM->SBUF (ACT copy / DVE copy), DMA slice to HBM
  6. exp(match/TEMP - 50) on ACT, fp16 out, accum_out = softmax partials
  7. u-weighted sums via fused scalar_tensor_tensor (u/512 fp16-exact)
v-sums use the per-chunk exp sums (v is constant within an image row).
"""

import contextlib
import os
import sys

sys.path.insert(0, "/opt/trn_rl_repo")

import numpy as np

B, C, H, W, N = 2, 128, 384, 512, 256
HWTOT = H * W              # 196608
NSHARD = 4                 # HW-axis shards per batch element
MSL = HWTOT // NSHARD      # 49152 columns per core
ROWS = H // NSHARD         # 96 image rows per core
QB = ROWS // 4             # sigma blocks of 4 rows
TEMP = 0.01
BOUND = 50.0               # fixed softmax bound: exp(match/TEMP - BOUND)
NCORES = 8
LOSHIFT = 1024.0           # lo-channel scaling for fp16 hi/lo sigma split


# ----------------------------------------------------------------------------
# host-side math (numpy mirrors of the jax reference ops)
# ----------------------------------------------------------------------------

def _bilinear_sample_np(fmap, kp):
    """fmap: [B,C,h,w] f32; kp: [B,n,2] pixel coords (u,v). Returns [B,C,n]."""
    b, c, h, w = fmap.shape
    u = np.clip(kp[..., 0], 0.0, w - 1.0).astype(np.float32)
    v = np.clip(kp[..., 1], 0.0, h - 1.0).astype(np.float32)
    x0f = np.floor(u)
    y0f = np.floor(v)
    wx = (u - x0f)[:, None, :]
    wy = (v - y0f)[:, None, :]
    x0 = x0f.astype(np.int64)
    y0 = y0f.astype(np.int64)
    x1 = np.minimum(x0 + 1, w - 1)
    y1 = np.minimum(y0 + 1, h - 1)
    fm = fmap.reshape(b, c, h * w)

    def g(yi, xi):
        idx = (yi * w + xi)[:, None, :]
        idx = np.broadcast_to(idx, (b, c, yi.shape[1]))
        return np.take_along_axis(fm, idx, axis=2)

    top = g(y0, x0) * (1.0 - wx) + g(y0, x1) * wx
    bot = g(y1, x0) * (1.0 - wx) + g(y1, x1) * wx
    return (top * (1.0 - wy) + bot * wy).astype(np.float32)


def _zncc_np(desc):
    mean = desc.mean(axis=1, keepdims=True)
    std = desc.std(axis=1, keepdims=True, ddof=1)
    return ((desc - mean) / std).astype(np.float32)


# ----------------------------------------------------------------------------
# bass program (built once per process)
# ----------------------------------------------------------------------------

_CACHE = {}


def _build_program():
    import concourse.tile as tile
    from concourse import bacc, mybir

    f32 = mybir.dt.float32
    f16 = mybir.dt.float16
    AF = mybir.ActivationFunctionType
    OP = mybir.AluOpType
    AX = mybir.AxisListType

    nc = bacc.Bacc("TRN2", target_bir_lowering=False, debug=False)

    fsl = nc.declare_dram_parameter("fsl", [C, ROWS, W], f32, isOutput=False)
    dsrc = nc.declare_dram_parameter("dsrc", [C, N], f32, isOutput=False)
    ub = nc.declare_dram_parameter("ub", [128, W], f16, isOutput=False)
    vb = nc.declare_dram_parameter("vb", [128, ROWS], f32, isOutput=False)
    sig = nc.declare_dram_parameter("sig", [8, QB * W], f16, isOutput=False)
    sel = nc.declare_dram_parameter("sel", [8, 512], f16, isOutput=False)
    match = nc.declare_dram_parameter("match", [N, MSL], f32, isOutput=True)
    acc = nc.declare_dram_parameter("acc", [2, 128, 3], f32, isOutput=True)

    with tile.TileContext(nc) as tc, contextlib.ExitStack() as ctx:
        ep = ctx.enter_context

        const_p = ep(tc.tile_pool(name="const", bufs=1))
        sums_p = ep(tc.tile_pool(name="sums", bufs=1))

        dsrc_sb = const_p.tile([C, N], f32, tag="dsrc")
        ub_sb = const_p.tile([128, W], f16, tag="ub")
        vb_sb = const_p.tile([128, ROWS], f32, tag="vb")
        sig_sb = const_p.tile([8, QB * W], f16, tag="sig")
        sel_sb = const_p.tile([8, 512], f16, tag="sel")
        biasb = const_p.tile([128, 1], f32, tag="biasb")
        nc.gpsimd.memset(biasb[:], -BOUND)
        nc.sync.dma_start(dsrc_sb[:], dsrc[:])
        nc.sync.dma_start(ub_sb[:], ub[:])
        nc.sync.dma_start(vb_sb[:], vb[:])
        nc.sync.dma_start(sig_sb[:], sig[:])
        nc.sync.dma_start(sel_sb[:], sel[:])

        esum = [
            sums_p.tile([128, ROWS], f32, tag=f"esum{r}", name=f"esum{r}")
            for r in range(2)
        ]
        usum = [
            sums_p.tile([128, ROWS], f32, tag=f"usum{r}", name=f"usum{r}")
            for r in range(2)
        ]

        f_p = ep(tc.tile_pool(name="f", bufs=6))
        fn_p = ep(tc.tile_pool(name="fn", bufs=3))
        msb_p = ep(tc.tile_pool(name="msb", bufs=6))
        exp_p = ep(tc.tile_pool(name="expb", bufs=4))
        uscr_p = ep(tc.tile_pool(name="uscr", bufs=4))
        mm_p = ep(tc.tile_pool(name="mm", bufs=4, space="PSUM"))
        bc_p = ep(tc.tile_pool(name="bc", bufs=2, space="PSUM"))

        for j in range(ROWS):
            p, q = j % 4, j // 4
            ft = f_p.tile([C, W], f32, tag="f")
            nc.sync.dma_start(ft[:], fsl[:, j, :])

            # sigma_bcast[128, 512] = hi_row_p + lo_row_p * 2^-10 (exact)
            bc_t = bc_p.tile([128, W], f32, tag="bc")
            nc.tensor.matmul(
                bc_t[:],
                sel_sb[:, 128 * p : 128 * (p + 1)],
                sig_sb[:, q * W : (q + 1) * W],
            )
            fn_t = fn_p.tile([C, W], f32, tag="fn")
            nc.vector.tensor_tensor(fn_t[:], ft[:], bc_t[:], op=OP.mult)

            for r in range(2):
                mm_t = mm_p.tile([128, W], f32, tag="mm")
                nc.tensor.matmul(
                    mm_t[:], dsrc_sb[:, 128 * r : 128 * (r + 1)], fn_t[:]
                )
                msb_t = msb_p.tile([128, W], f32, tag="msb")
                nc.scalar.copy(msb_t[:], mm_t[:])
                nc.sync.dma_start(
                    match[128 * r : 128 * (r + 1), j * W : (j + 1) * W],
                    msb_t[:],
                )
                exp_t = exp_p.tile([128, W], f32, tag="expb")
                nc.scalar.activation(
                    exp_t[:], mm_t[:], AF.Exp,
                    bias=biasb[:], scale=1.0 / TEMP,
                    accum_out=esum[r][:, j : j + 1],
                )
                uscr_t = uscr_p.tile([128, W], f32, tag="uscr")
                nc.vector.scalar_tensor_tensor(
                    out=uscr_t[:], in0=exp_t[:], scalar=1.0,
                    in1=ub_sb[:], op0=OP.mult, op1=OP.mult,
                    accum_out=usum[r][:, j : j + 1],
                )

        # finale: totals -> acc [2, 128, 3] = (sum_e, sum_e*u/512, sum_e*v)
        acc_sb = sums_p.tile([128, 8], f32, tag="accsb")
        vscr = sums_p.tile([128, ROWS], f32, tag="vscr")
        for r in range(2):
            nc.vector.tensor_reduce(
                acc_sb[:, 4 * r : 4 * r + 1], esum[r][:], axis=AX.X, op=OP.add
            )
            nc.vector.tensor_reduce(
                acc_sb[:, 4 * r + 1 : 4 * r + 2], usum[r][:], axis=AX.X,
                op=OP.add,
            )
            nc.vector.scalar_tensor_tensor(
                out=vscr[:], in0=esum[r][:], scalar=1.0, in1=vb_sb[:],
                op0=OP.mult, op1=OP.mult,
                accum_out=acc_sb[:, 4 * r + 2 : 4 * r + 3],
            )
        for r in range(2):
            nc.sync.dma_start(acc[r, :, :], acc_sb[:, 4 * r : 4 * r + 3])

    nc.compile()
    return nc


def _get_program():
    if "nc" not in _CACHE:
        _CACHE["nc"] = _build_program()
    return _CACHE["nc"]


def _make_inmaps(fm_trg, desc_src):
    """Build the 8 per-core input maps (host prep: slicing + sigma hi/lo)."""
    ubv = np.arange(W, dtype=np.float32) / float(W)
    ub_np = np.broadcast_to(ubv.astype(np.float16), (128, W)).copy()
    # selector: rows 0-3 pick the hi row p; rows 4-7 add lo row p * 2^-10
    sel_np = np.zeros((8, 512), dtype=np.float16)
    for p in range(4):
        sel_np[p, 128 * p : 128 * (p + 1)] = 1.0
        sel_np[4 + p, 128 * p : 128 * (p + 1)] = 1.0 / LOSHIFT

    # sigma^-1/C for all positions, per batch: [B, HWTOT]
    fm = fm_trg.reshape(B, C, HWTOT).astype(np.float32)
    std = fm.std(axis=1, ddof=1)
    siginv = (1.0 / (std * float(C))).astype(np.float32)  # [B, HWTOT]

    in_maps = []
    for core in range(NCORES):
        b, qq = divmod(core, NSHARD)
        sl = siginv[b][qq * MSL : (qq + 1) * MSL].reshape(QB, 4, W)
        hi = sl.astype(np.float16)                      # [QB, 4, W]
        lo = ((sl - hi.astype(np.float32)) * LOSHIFT).astype(np.float16)
        sig_np = np.empty((8, QB * W), dtype=np.float16)
        sig_np[0:4] = hi.transpose(1, 0, 2).reshape(4, QB * W)
        sig_np[4:8] = lo.transpose(1, 0, 2).reshape(4, QB * W)
        vbv = qq * ROWS + np.arange(ROWS, dtype=np.float32)
        in_maps.append({
            "fsl": np.ascontiguousarray(
                fm_trg[b].reshape(C, H, W)[:, qq * ROWS : (qq + 1) * ROWS, :]
            ),
            "dsrc": np.ascontiguousarray(desc_src[b]),
            "ub": ub_np,
            "vb": np.broadcast_to(vbv, (128, ROWS)).copy(),
            "sig": sig_np,
            "sel": sel_np,
        })
    return in_maps


# ----------------------------------------------------------------------------
# entry point
# ----------------------------------------------------------------------------

def kernel(**inputs):
    kp_src = np.asarray(inputs["keypoints_2D_src"], dtype=np.float32)
    kp_trg = np.asarray(inputs["keypoints_2D_trg"], dtype=np.float32)
    fm_src = np.asarray(inputs["feature_map_src"], dtype=np.float32)
    fm_trg = np.asarray(inputs["feature_map_trg"], dtype=np.float32)
    sc_src = np.asarray(inputs["scores_map_src"], dtype=np.float32)
    sc_trg = np.asarray(inputs["scores_map_trg"], dtype=np.float32)
    use_weights = int(inputs["use_weights"])
    all_trg_points = int(inputs["all_trg_points"])

    desc_src = _zncc_np(_bilinear_sample_np(fm_src, kp_src))  # [B,C,N]

    if not all_trg_points:
        # tiny problem (M = N = 256): plain numpy is exact and fast
        desc_trg = _zncc_np(_bilinear_sample_np(fm_trg, kp_trg))
        match_vals = np.einsum("bcn,bcm->bnm", desc_src, desc_trg) / float(C)
        x = match_vals / TEMP
        x = x - x.max(axis=2, keepdims=True)
        e = np.exp(x)
        soft = e / e.sum(axis=2, keepdims=True)
        kp_pseudo = np.einsum("bnm,bmd->bnd", soft, kp_trg).astype(np.float32)
    else:
        match_vals, kp_pseudo = _device_match(fm_trg, desc_src)

    desc_pseudo = _zncc_np(_bilinear_sample_np(fm_trg, kp_pseudo))
    mvp = np.einsum("bcn,bcn->bn", desc_src, desc_pseudo) / float(C)
    mvp = mvp.astype(np.float32)
    match_val_pairs = mvp[:, None, :]
    weights = scores_src = scores_pseudo = None
    if use_weights:
        weights = 0.5 * (match_val_pairs + 1.0)
        scores_src = _bilinear_sample_np(sc_src, kp_src)
        scores_pseudo = _bilinear_sample_np(sc_trg, kp_pseudo)
        weights = (weights * scores_src * scores_pseudo).astype(np.float32)
    return (kp_pseudo, weights, scores_src, scores_pseudo, match_vals, mvp)


def _device_match(fm_trg, desc_src):
    """Run the heavy part (match_vals + softargmax partials) on 8 cores."""
    from concourse.bass_utils import run_bass_kernel_spmd

    nc = _get_program()
    in_maps = _make_inmaps(fm_trg, desc_src)

    kres = run_bass_kernel_spmd(
        nc, in_maps, list(range(NCORES)),
        trace=bool(int(os.environ.get("KMV_TRACE", "0"))),
    )
    if kres.exec_time_ns is not None:
        _CACHE["exec_time_ns"] = kres.exec_time_ns
    res = kres.results

    match_vals = np.empty((B, N, HWTOT), dtype=np.float32)
    etot = np.zeros((B, N), dtype=np.float64)
    utot = np.zeros((B, N), dtype=np.float64)
    vtot = np.zeros((B, N), dtype=np.float64)
    for core in range(NCORES):
        b, q = divmod(core, NSHARD)
        out = res[core]
        match_vals[b][:, q * MSL : (q + 1) * MSL] = out["match"]
        a = out["acc"]  # [2, 128, 3]
        for r in range(2):
            sl = slice(128 * r, 128 * (r + 1))
            etot[b][sl] += a[r, :, 0].astype(np.float64)
            utot[b][sl] += a[r, :, 1].astype(np.float64)
            vtot[b][sl] += a[r, :, 2].astype(np.float64)

    pu = (utot * float(W)) / etot
    pv = vtot / etot
    kp_pseudo = np.stack([pu, pv], axis=2).astype(np.float32)
    return match_vals, kp_pseudo
